# revision 2
# baseline (speedup 1.0000x reference)
"""DiT block with MoE — Trainium2 Bass/Tile kernel, 8-core SPMD, v2.

v2 strategy (vs v1's token-sharded attention + big K/V AllGather):
  - Self-attention is HEAD-sharded inside each 4-core batch group: core r
    computes k/q/v for its 3 heads over ALL 2048 tokens of its batch, runs
    attention locally (K/V never leave the core), then combines the
    o-projection with a dim-major ReduceScatter (1.5MB out) instead of
    gathering 15.7MB of K/V before any attention work can start.
  - The q/k rmsnorm needs full-dim sums; a 24KB AllReduce of per-token
    partial sum-of-squares provides them (hidden behind the v projection).
  - Cross-attention K/V are computed head-sharded over the full 512-token
    context and AllGathered (3MB) during self-attention, so cross-attention
    (token-sharded, all heads for the core's own 512 tokens) never waits.
  - The residual stream lives in SBUF end-to-end, updated in place (v1
    bounced it through DRAM between stages, serializing layernorms behind
    3MB roundtrips).
  - ln1 runs on bf16 inputs directly (no f32->bf16 copies) since every
    downstream consumer is a bf16 matmul.
  - MoE stays dense-masked: host computes per-token expert mask rows;
    each expert output is (psum + b_e) * mask_e fused on DVE, accumulated.

SBUF is budgeted to ~205KB/partition via aggressive slot sharing (the hff
slot holds h early and the FFN activations late; k/q/v SBUF blocks are
recycled for cross-attention tensors once self-attention drains).
"""

import contextlib
import os

import numpy as np
import ml_dtypes

import concourse.bass as bass
import concourse.tile as tile
import concourse.mybir as mybir
from concourse import bacc, bass_utils
from concourse.bass import ds, ts

B, S, CTX, DIM, NH, FFN, NE, TOPK = 2, 2048, 512, 1536, 12, 6144, 4, 2
HD = DIM // NH          # 128
EPS = 1e-6
N_CORES = 8
CPB = N_CORES // B      # cores per batch group = 4
TPC = S // CPB          # own tokens per core = 512
KC = DIM // 128         # dim chunks = 12
HC = NH // CPB          # head chunks per core = 3
HDW = HC * HD           # head-shard width = 384
FC = FFN // 128         # ffn chunks = 48
NTC = S // TPC          # token chunks = 4
CTC = CTX // 128        # ctx chunks = 4
BF16 = mybir.dt.bfloat16
F32 = mybir.dt.float32
bfnp = ml_dtypes.bfloat16

ACTF = mybir.ActivationFunctionType
ALU = mybir.AluOpType

# AllGather#2 block: ck (post-rms) [3,128,CTX] + cv [4tw,128,384]
SZ_CKB = HC * 128 * CTX
SZ_CVB = CTC * 128 * HDW
BLK2 = SZ_CKB + SZ_CVB

_VEC = [
    ("s1p", KC), ("sh1", KC), ("g1", KC), ("s2p", KC), ("sh2", KC),
    ("g2", KC), ("qb", HC), ("kb", HC), ("nqw", HC), ("nkw", HC),
    ("ob", KC), ("cqb", KC), ("ckb", HC), ("cnqw", KC), ("cnkw", HC),
    ("cob", KC), ("n3w", KC), ("n3b", KC), ("b2", KC),
    ("mb0", KC), ("mb1", KC), ("mb2", KC), ("mb3", KC), ("fb1", FC),
]
_VBASE = {}
_off = 0
for _n, _c in _VEC:
    _VBASE[_n] = _off
    _off += _c
NV = _off


def build_bass(debug=False, stop_stage=99):
    nc = bacc.Bacc("TRN2", target_bir_lowering=False, debug=False,
                   num_devices=N_CORES)

    def din(name, shape, dt):
        return nc.dram_tensor(name, shape, dt, kind="ExternalInput").ap()

    io = dict(
        xallT=din("xallT", [DIM, S], BF16),
        xT=din("xT", [DIM, TPC], F32),
        ctxT=din("ctxT", [DIM, CTX], BF16),
        cosT=din("cosT", [HD, S], BF16),
        sinT=din("sinT", [HD, S], BF16),
        vecs_d=din("vecs", [128, NV], F32),
        rowb_d=din("rowb", [1, 2 * HDW], F32),
        wq=din("wq", [HC, 128, DIM], BF16),
        wk=din("wk", [HC, 128, DIM], BF16),
        wv=din("wv", [DIM, HDW], BF16),
        wo=din("wo", [KC, 128, HDW], BF16),
        cwq=din("cwq", [KC, 128, DIM], BF16),
        cwk=din("cwk", [HC, 128, DIM], BF16),
        cwv=din("cwv", [DIM, HDW], BF16),
        cwo=din("cwo", [KC, 128, DIM], BF16),
        w1=din("w1", [FC, 128, DIM], BF16),
        w2=din("w2", [KC, 128, FFN], BF16),
        moew=din("moew", [NE, KC, 128, DIM], BF16),
        wall_d=din("wall", [NE, TPC], F32),
        yT=nc.dram_tensor("yT", [DIM, TPC], F32, kind="ExternalOutput").ap(),
    )

    with tile.TileContext(nc) as tc:
        _emit(nc, tc, io, stop_stage)
    nc.compile()
    return nc


def _emit(nc, tc, io, stop_stage):
    xallT, xT, ctxT, cosT, sinT = (io["xallT"], io["xT"], io["ctxT"],
                                   io["cosT"], io["sinT"])
    vecs_d, rowb_d = io["vecs_d"], io["rowb_d"]
    wq, wk, wv, wo = io["wq"], io["wk"], io["wv"], io["wo"]
    cwq, cwk, cwv, cwo = io["cwq"], io["cwk"], io["cwv"], io["cwo"]
    w1, w2, moew, wall_d = io["w1"], io["w2"], io["moew"], io["wall_d"]
    yT = io["yT"]

    ctx = contextlib.ExitStack()
    const = ctx.enter_context(tc.tile_pool(name="const", bufs=1))
    xa = ctx.enter_context(tc.tile_pool(name="xa", bufs=2))      # 12K slots
    xr = ctx.enter_context(tc.tile_pool(name="xr", bufs=1))      # resid f32
    hp = ctx.enter_context(tc.tile_pool(name="hp", bufs=1))      # h / ff
    hs = ctx.enter_context(tc.tile_pool(name="hs", bufs=1))      # h3 / h2
    kvp = ctx.enter_context(tc.tile_pool(name="kvp", bufs=1))    # v -> cvb
    raw = ctx.enter_context(tc.tile_pool(name="raw", bufs=1))    # k/q raw
    cxp = ctx.enter_context(tc.tile_pool(name="cxp", bufs=1))    # ck raw
    wp = ctx.enter_context(tc.tile_pool(name="wp", bufs=2))      # stream wts
    wp2 = ctx.enter_context(tc.tile_pool(name="wp2", bufs=2))
    sm = ctx.enter_context(tc.tile_pool(name="sm", bufs=3))
    att = ctx.enter_context(tc.tile_pool(name="att", bufs=1))    # attn/cq
    exq = ctx.enter_context(tc.tile_pool(name="exq", bufs=2))
    tmp = ctx.enter_context(tc.tile_pool(name="tmp", bufs=3))
    pin = ctx.enter_context(tc.tile_pool(name="pin", bufs=2))    # mb/rb rows
    vcp = ctx.enter_context(tc.tile_pool(name="vcp", bufs=1))
    psA = ctx.enter_context(tc.tile_pool(name="psA", bufs=4, space="PSUM"))
    psN = ctx.enter_context(tc.tile_pool(name="psN", bufs=2, space="PSUM"))
    psB = ctx.enter_context(tc.tile_pool(name="psB", bufs=2, space="PSUM"))
    dram = ctx.enter_context(tc.tile_pool(name="dram", bufs=1, space="DRAM"))

    # ---------------- constants
    vecs = const.tile([128, NV], F32, name="vecs_s")
    nc.sync.dma_start(vecs[:], vecs_d[:])

    def vcol(name, c):
        i = _VBASE[name] + c
        return vecs[:, i:i + 1]

    rowb_bf = const.tile([1, 2 * HDW], BF16, name="rowb_bf")
    nc.gpsimd.dma_start(rowb_bf[:], rowb_d[:])      # casting DMA
    cos_t = const.tile([HD, S], BF16, name="cos_s")
    nc.sync.dma_start(cos_t[:], cosT[:])
    sin_t = const.tile([HD, S], BF16, name="sin_s")
    nc.sync.dma_start(sin_t[:], sinT[:])
    ones_c = const.tile([128, 1], BF16, name="ones_c")
    nc.vector.memset(ones_c[:], 1.0)
    ones_r = const.tile([1, 128], BF16, name="ones_r")
    nc.vector.memset(ones_r[:], 1.0)
    ones_rf = const.tile([1, 128], F32, name="ones_rf")
    nc.vector.memset(ones_rf[:], 1.0)
    eps_t = const.tile([1, 1], F32, name="eps_t")
    nc.vector.memset(eps_t[:], EPS)

    SCL = float(1.0 / np.sqrt(HD))

    # DRAM scratch
    arb = dram.tile([3, S], F32, name="arb")            # rms partials
    arbo = dram.tile([3, S], F32, name="arbo")          # reduced result
    aginb = dram.tile([BLK2], BF16, name="aginb")       # cross k/v bounce
    agout = dram.tile([CPB, BLK2], BF16, name="agout")
    rsin = dram.tile([CPB, DIM, TPC], BF16, name="rsin")  # rank-block-major
    rsout = dram.tile([DIM, TPC], BF16, name="rsout")

    groups = [list(range(CPB)), list(range(CPB, 2 * CPB))]

    # ---------------- helpers
    def bcast_row(row_ap, n_tok):
        ps = psB.tile([128, n_tok], F32, tag="bc", name="bc_ps")
        nc.tensor.matmul(ps[:], ones_rf[:], row_ap, start=True, stop=True,
                         skip_group_check=True)
        return ps

    def stop_dump(chunks):
        for c, tl in enumerate(chunks):
            tf = tmp.tile([128, TPC], F32, tag="f32t", name="stopf")
            nc.vector.tensor_copy(tf[:], tl)
            nc.sync.dma_start(yT[ts(c, 128), :], tf[:])
        ctx.close()

    # ================= stage 1: ln1 over all S tokens -> h (bf16, in SBUF)
    h = hp.tile([128, KC, S], BF16, tag="hff", name="h")
    for tci in range(NTC):
        xb = xa.tile([128, KC, TPC], BF16, tag="xall", name="xac")
        nc.scalar.dma_start(
            xb[:],
            bass.AP(tensor=xallT.tensor, offset=xallT.offset + tci * TPC,
                    ap=[[S, 128], [128 * S, KC], [1, TPC]]))
        ps_s = psN.tile([1, TPC], F32, tag="nsum", name="ln_ps_s")
        ps_q = psN.tile([1, TPC], F32, tag="nsum", name="ln_ps_q")
        for c in range(KC):
            nc.tensor.matmul(ps_s[:], ones_c[:], xb[:, c, :], start=(c == 0),
                             stop=(c == KC - 1), skip_group_check=True)
            sq = tmp.tile([128, TPC], BF16, tag="bft", name="lnsq")
            nc.gpsimd.tensor_mul(sq[:], xb[:, c, :], xb[:, c, :])
            nc.tensor.matmul(ps_q[:], ones_c[:], sq[:], start=(c == 0),
                             stop=(c == KC - 1), skip_group_check=True)
        mean = sm.tile([1, TPC], F32, tag="s", name="mean")
        nc.scalar.activation(mean[:], ps_s[:], ACTF.Copy, scale=1.0 / DIM)
        ex2 = sm.tile([1, TPC], F32, tag="s", name="ex2")
        nc.scalar.activation(ex2[:], ps_q[:], ACTF.Copy, scale=1.0 / DIM)
        m2 = sm.tile([1, TPC], F32, tag="s", name="m2")
        nc.vector.tensor_mul(m2[:], mean[:], mean[:])
        var = sm.tile([1, TPC], F32, tag="s", name="var")
        nc.vector.tensor_sub(var[:], ex2[:], m2[:])
        std = sm.tile([1, TPC], F32, tag="s", name="std")
        nc.scalar.activation(std[:], var[:], ACTF.Sqrt, bias=eps_t[:1, :])
        rstd = sm.tile([1, TPC], F32, tag="s", name="rstd")
        nc.vector.reciprocal(rstd[:], std[:])
        mb_ps = bcast_row(mean[:], TPC)
        rb_ps = bcast_row(rstd[:], TPC)
        mb = pin.tile([128, TPC], BF16, tag="pinb", name="mb")
        nc.vector.tensor_copy(mb[:], mb_ps[:])
        rb = pin.tile([128, TPC], BF16, tag="pinb", name="rb")
        nc.vector.tensor_copy(rb[:], rb_ps[:])
        for c in range(KC):
            u = tmp.tile([128, TPC], BF16, tag="bft", name="lnu")
            nc.vector.tensor_sub(u[:], xb[:, c, :], mb[:])
            u2 = tmp.tile([128, TPC], BF16, tag="bft", name="lnu2")
            nc.vector.tensor_mul(u2[:], u[:], rb[:])
            nc.scalar.activation(h[:, c, ts(tci, TPC)], u2[:], ACTF.Identity,
                                 bias=vcol("sh1", c), scale=vcol("s1p", c))
    if stop_stage <= 1:
        return stop_dump([h[:, c, ts(0, TPC)] for c in range(KC)])

    xown = xr.tile([128, KC, TPC], F32, tag="xo", name="xown")
    nc.scalar.dma_start(
        xown[:], xT[:, :].rearrange("(c p) t -> p c t", p=128))

    # ================= stage 2: k, q, ck projections (head shard)
    def shard_proj(w_d, bias, n_tok, out_big, x_chunks):
        for o in range(HC):
            wt = wp.tile([128, DIM], BF16, tag="w", name="wt")
            nc.scalar.dma_start(wt[:], w_d[o])
            for tci in range(n_tok // TPC):
                ps = psA.tile([128, TPC], F32, tag="mm", name="proj_ps")
                for k in range(KC):
                    nc.tensor.matmul(ps[:], wt[:, ts(k, 128)],
                                     x_chunks(k, tci),
                                     start=(k == 0), stop=(k == KC - 1),
                                     skip_group_check=True)
                nc.scalar.activation(out_big[:, o, ts(tci, TPC)], ps[:],
                                     ACTF.Identity, bias=bias(o))

    h_ch = lambda k, tci: h[:, k, ts(tci, TPC)]

    def rms_apply_big(big, n_tok, row, wname):
        """in-place: big <- big * rsqrt(mean_full(big^2)+eps) * w."""
        for tci in range(n_tok // TPC):
            srow = sm.tile([1, TPC], F32, tag="s", name="rms_in")
            nc.scalar.dma_start(srow[:], arbo[row:row + 1, ts(tci, TPC)])
            ms = sm.tile([1, TPC], F32, tag="s", name="rms_ms")
            nc.scalar.activation(ms[:], srow[:], ACTF.Sqrt, bias=eps_t[:1, :],
                                 scale=1.0 / DIM)
            rstd = sm.tile([1, TPC], F32, tag="s", name="rms_r")
            nc.vector.reciprocal(rstd[:], ms[:])
            rb_ps = bcast_row(rstd[:], TPC)
            rb = pin.tile([128, TPC], BF16, tag="pinb", name="rms_rb")
            nc.vector.tensor_copy(rb[:], rb_ps[:])
            for c in range(HC):
                u = tmp.tile([128, TPC], BF16, tag="bft", name="rms_u")
                nc.vector.tensor_mul(u[:], big[:, c, ts(tci, TPC)], rb[:])
                nc.scalar.activation(big[:, c, ts(tci, TPC)], u[:],
                                     ACTF.Identity, scale=vcol(wname, c))

    def rope_big(big, n_tok):
        for c in range(HC):
            for tci in range(n_tok // TPC):
                q = big[:, c, ts(tci, TPC)]
                qs = tmp.tile([128, TPC], BF16, tag="bft", name="rpswap")
                nc.scalar.dma_start(qs[0:64, :], big[64:128, c, ts(tci, TPC)])
                nc.scalar.dma_start(qs[64:128, :], big[0:64, c, ts(tci, TPC)])
                t1 = tmp.tile([128, TPC], BF16, tag="bft", name="rp1")
                nc.vector.tensor_mul(t1[:], q, cos_t[:, ts(tci, TPC)])
                t2 = tmp.tile([128, TPC], BF16, tag="bft", name="rp2")
                nc.vector.tensor_mul(t2[:], qs[:], sin_t[:, ts(tci, TPC)])
                nc.vector.tensor_add(q, t1[:], t2[:])


    k_raw = raw.tile([128, HC, S], BF16, tag="kr", name="k_raw")
    shard_proj(wk, lambda o: vcol("kb", o), S, k_raw, h_ch)

    def sumsq_row(big, n_tok, row):
        """partial sum over my HC chunks of big^2 -> arb[row, :n_tok]."""
        for tci in range(n_tok // TPC):
            ps_q = psN.tile([1, TPC], F32, tag="nsum", name="rms_ps")
            for c in range(HC):
                sq = tmp.tile([128, TPC], BF16, tag="bft", name="rmsq")
                nc.gpsimd.tensor_mul(sq[:], big[:, c, ts(tci, TPC)],
                                     big[:, c, ts(tci, TPC)])
                nc.tensor.matmul(ps_q[:], ones_c[:], sq[:], start=(c == 0),
                                 stop=(c == HC - 1), skip_group_check=True)
            srow = sm.tile([1, TPC], F32, tag="s", name="ssq")
            nc.vector.tensor_copy(srow[:], ps_q[:])
            nc.sync.dma_start(arb[row:row + 1, ts(tci, TPC)], srow[:])

    sumsq_row(k_raw, S, 0)
    rope_big(k_raw, S)

    q_raw = raw.tile([128, HC, S], BF16, tag="qr", name="q_raw")
    shard_proj(wq, lambda o: vcol("qb", o), S, q_raw, h_ch)
    sumsq_row(q_raw, S, 1)
    rope_big(q_raw, S)

    ctxb = xa.tile([128, KC, CTX], BF16, tag="xall", name="ctxb")
    nc.sync.dma_start(ctxb[:], ctxT[:, :].rearrange("(c p) t -> p c t", p=128))
    ck_raw = cxp.tile([128, HC, CTX], BF16, tag="ckr", name="ck_raw")
    shard_proj(cwk, lambda o: vcol("ckb", o), CTX, ck_raw,
               lambda k, tci: ctxb[:, k, :])
    sumsq_row(ck_raw, CTX, 2)

    # ================= stage 3: AllReduce rms partials (in-place on arb)
    nc.gpsimd.collective_compute(
        "AllReduce", ALU.add, replica_groups=groups,
        ins=[arb.opt()], outs=[arbo.opt()])

    # ================= stage 4: v / cv projections (overlap AllReduce)
    def tokmajor_shard(x_chunks, w_d, n_tok, bias_off, store):
        """store(tw, ps) with ps = [128tok, HDW] = x^T W + rowb."""
        n_tw = n_tok // 128
        for g in range(0, n_tw, 4):
            gtw = list(range(g, min(g + 4, n_tw)))
            pss = [psA.tile([128, HDW], F32, tag="mm", name=f"vps{i}")
                   for i in range(len(gtw))]
            for k in range(KC):
                wvb = wp.tile([128, HDW], BF16, tag="w", name="wvb")
                nc.scalar.dma_start(wvb[:], w_d[ts(k, 128), :])
                for i, tw in enumerate(gtw):
                    nc.tensor.matmul(pss[i][:], x_chunks(k, tw), wvb[:],
                                     start=(k == 0), stop=False,
                                     skip_group_check=True)
            for i, tw in enumerate(gtw):
                nc.tensor.matmul(pss[i][:], ones_r[:],
                                 rowb_bf[:, ds(bias_off, HDW)],
                                 start=False, stop=True, skip_group_check=True)
                store(tw, pss[i])

    v_sb = kvp.tile([128, S // 128, HDW], BF16, tag="vsb", name="v_sb")

    def v_store(tw, ps):
        nc.vector.tensor_copy(v_sb[:, tw, :], ps[:])

    tokmajor_shard(lambda k, tw: h[:, k, ts(tw, 128)], wv, S, 0, v_store)

    def cv_store(tw, ps):
        vt = vcp.tile([128, HDW], BF16, tag="vc", name="cvtmp")
        nc.vector.tensor_copy(vt[:], ps[:])
        nc.sync.dma_start(
            bass.AP(tensor=aginb.tensor,
                    offset=aginb.offset + SZ_CKB + tw * 128 * HDW,
                    ap=[[HDW, 128], [1, HDW]]),
            vt[:])

    tokmajor_shard(lambda k, tw: ctxb[:, k, ts(tw, 128)], cwv, CTX,
                   HDW, cv_store)

    # ================= stage 5: rms-apply k,q; ck; bounce ck
    rms_apply_big(k_raw, S, 0, "nkw")
    kT = k_raw
    rms_apply_big(q_raw, S, 1, "nqw")
    qT = q_raw
    if stop_stage <= 4:
        return stop_dump([qT[:, c % HC, ts(c // HC, TPC)] for c in range(KC)])

    rms_apply_big(ck_raw, CTX, 2, "cnkw")
    for c in range(HC):
        nc.sync.dma_start(
            aginb[ds(c * 128 * CTX, 128 * CTX)].rearrange("(p t) -> p t", p=128),
            ck_raw[:, c, :])

    # ================= stage 6: AllGather cross k/v (overlaps self-attn)
    nc.gpsimd.collective_compute(
        "AllGather", ALU.bypass, replica_groups=groups,
        ins=[aginb.opt()], outs=[agout.opt()])

    # ================= stage 7: self-attention (local, head-sharded)
    attnb = att.tile([128, HC, S], BF16, tag="attnb", name="attnb")
    for hh in range(HC):
        for tci in range(NTC):
            aps = psA.tile([128, TPC], F32, tag="mm", name="aps")
            dps = psN.tile([1, TPC], F32, tag="nsum", name="dps")
            for j in range(S // 128):
                sps = psA.tile([128, TPC], F32, tag="mm", name="sps")
                nc.tensor.matmul(sps[:], kT[:, hh, ts(j, 128)],
                                 qT[:, hh, ts(tci, TPC)],
                                 start=True, stop=True, skip_group_check=True)
                ex = exq.tile([128, TPC], BF16, tag="ex", name="ex")
                nc.scalar.activation(ex[:], sps[:], ACTF.Exp, scale=SCL)
                nc.tensor.matmul(aps[:], v_sb[:, j, ts(hh, 128)], ex[:],
                                 start=(j == 0), stop=(j == S // 128 - 1),
                                 skip_group_check=True)
                nc.tensor.matmul(dps[:], ones_c[:], ex[:],
                                 start=(j == 0), stop=(j == S // 128 - 1),
                                 skip_group_check=True)
            rec = sm.tile([1, TPC], F32, tag="s", name="rec")
            nc.vector.reciprocal(rec[:], dps[:])
            rb_ps = bcast_row(rec[:], TPC)
            rb = tmp.tile([128, TPC], F32, tag="f32t", name="arb_t")
            nc.vector.tensor_copy(rb[:], rb_ps[:])
            nc.vector.tensor_mul(attnb[:, hh, ts(tci, TPC)], aps[:], rb[:])

    # ================= stage 8: partial o-proj -> rsin (dim-major)
    for o in range(KC):
        wt = wp.tile([128, HDW], BF16, tag="w", name="wot")
        nc.sync.dma_start(wt[:], wo[o])
        for tci in range(NTC):
            ps = psA.tile([128, TPC], F32, tag="mm", name="ops")
            for c in range(HC):
                nc.tensor.matmul(ps[:], wt[:, ts(c, 128)],
                                 attnb[:, c, ts(tci, TPC)],
                                 start=(c == 0), stop=(c == HC - 1),
                                 skip_group_check=True)
            ot = vcp.tile([128, TPC], BF16, tag="vc", name="otmp")
            nc.vector.tensor_copy(ot[:], ps[:])
            nc.sync.dma_start(
                bass.AP(tensor=rsin.tensor,
                        offset=rsin.offset + (tci * DIM + o * 128) * TPC,
                        ap=[[TPC, 128], [1, TPC]]),
                ot[:])

    # ================= stage 9: ReduceScatter partial-o -> rsout
    nc.gpsimd.collective_compute(
        "ReduceScatter", ALU.add, replica_groups=groups,
        ins=[rsin.opt()], outs=[rsout.opt()])

    # ================= stage 10: x <- x + g1*(o + ob)   (in place, SBUF)
    ob_sb = xa.tile([128, KC, TPC], BF16, tag="xall", name="ob_sb")
    nc.scalar.dma_start(
        ob_sb[:], rsout[:, :].rearrange("(c p) t -> p c t", p=128))
    for c in range(KC):
        t1 = tmp.tile([128, TPC], F32, tag="f32t", name="ot1")
        nc.vector.tensor_scalar(t1[:], ob_sb[:, c, :], vcol("ob", c),
                                vcol("g1", c), ALU.add, ALU.mult)
        nc.vector.tensor_add(xown[:, c, :], t1[:], xown[:, c, :])
    if stop_stage <= 6:
        return stop_dump([xown[:, c, :] for c in range(KC)])

    # ================= stage 11: ln3 -> h3; cq (full dim, own tokens)
    def layernorm_own(xbig, out_big, sname, shname, wname, bname):
        ps_s = psN.tile([1, TPC], F32, tag="nsum", name="lno_s")
        ps_q = psN.tile([1, TPC], F32, tag="nsum", name="lno_q")
        xbf = []
        for c in range(KC):
            xb = tmp.tile([128, TPC], BF16, tag="bft", name="lnxb")
            nc.gpsimd.tensor_copy(xb[:], xbig[:, c, :])
            xbf.append(xb)
            nc.tensor.matmul(ps_s[:], ones_c[:], xb[:], start=(c == 0),
                             stop=(c == KC - 1), skip_group_check=True)
            sq = tmp.tile([128, TPC], BF16, tag="bft", name="lnsq2")
            nc.gpsimd.tensor_mul(sq[:], xb[:], xb[:])
            nc.tensor.matmul(ps_q[:], ones_c[:], sq[:], start=(c == 0),
                             stop=(c == KC - 1), skip_group_check=True)
            xbf[c] = None  # chunks re-read from xbig below; don't hold slots
        mean = sm.tile([1, TPC], F32, tag="s", name="mean3")
        nc.scalar.activation(mean[:], ps_s[:], ACTF.Copy, scale=1.0 / DIM)
        ex2 = sm.tile([1, TPC], F32, tag="s", name="ex23")
        nc.scalar.activation(ex2[:], ps_q[:], ACTF.Copy, scale=1.0 / DIM)
        m2 = sm.tile([1, TPC], F32, tag="s", name="m23")
        nc.vector.tensor_mul(m2[:], mean[:], mean[:])
        var = sm.tile([1, TPC], F32, tag="s", name="var3")
        nc.vector.tensor_sub(var[:], ex2[:], m2[:])
        std = sm.tile([1, TPC], F32, tag="s", name="std3")
        nc.scalar.activation(std[:], var[:], ACTF.Sqrt, bias=eps_t[:1, :])
        rstd = sm.tile([1, TPC], F32, tag="s", name="rstd3")
        nc.vector.reciprocal(rstd[:], std[:])
        mb_ps = bcast_row(mean[:], TPC)
        rb_ps = bcast_row(rstd[:], TPC)
        mb = pin.tile([128, TPC], BF16, tag="pinb", name="mb3")
        nc.vector.tensor_copy(mb[:], mb_ps[:])
        rb = pin.tile([128, TPC], BF16, tag="pinb", name="rb3")
        nc.vector.tensor_copy(rb[:], rb_ps[:])
        for c in range(KC):
            u = tmp.tile([128, TPC], F32, tag="f32t", name="lnu3")
            nc.vector.tensor_sub(u[:], xbig[:, c, :], mb[:])
            u2 = tmp.tile([128, TPC], F32, tag="f32t", name="lnu23")
            nc.vector.tensor_mul(u2[:], u[:], rb[:])
            if sname is not None:
                nc.scalar.activation(out_big[:, c, :], u2[:], ACTF.Identity,
                                     bias=vcol(shname, c), scale=vcol(sname, c))
            else:
                nc.scalar.activation(out_big[:, c, :], u2[:], ACTF.Identity,
                                     bias=vcol(bname, c), scale=vcol(wname, c))

    h3 = hs.tile([128, KC, TPC], BF16, tag="hh", name="h3")
    layernorm_own(xown, h3, None, None, "n3w", "n3b")

    cq_raw = att.tile([128, KC, TPC], BF16, tag="attnb", name="cq_raw")
    for o in range(KC):
        wt = wp.tile([128, DIM], BF16, tag="w", name="cwqt")
        nc.sync.dma_start(wt[:], cwq[o])
        ps = psA.tile([128, TPC], F32, tag="mm", name="cq_ps")
        for k in range(KC):
            nc.tensor.matmul(ps[:], wt[:, ts(k, 128)], h3[:, k, :],
                             start=(k == 0), stop=(k == KC - 1),
                             skip_group_check=True)
        nc.scalar.activation(cq_raw[:, o, :], ps[:], ACTF.Identity,
                             bias=vcol("cqb", o))
    # local full-dim rms for cq (in place)
    ps_q = psN.tile([1, TPC], F32, tag="nsum", name="cqrms")
    for c in range(KC):
        sq = tmp.tile([128, TPC], BF16, tag="bft", name="cqsq")
        nc.gpsimd.tensor_mul(sq[:], cq_raw[:, c, :], cq_raw[:, c, :])
        nc.tensor.matmul(ps_q[:], ones_c[:], sq[:], start=(c == 0),
                         stop=(c == KC - 1), skip_group_check=True)
    ms = sm.tile([1, TPC], F32, tag="s", name="cq_ms")
    nc.scalar.activation(ms[:], ps_q[:], ACTF.Sqrt, bias=eps_t[:1, :],
                         scale=1.0 / DIM)
    rstd = sm.tile([1, TPC], F32, tag="s", name="cq_r")
    nc.vector.reciprocal(rstd[:], ms[:])
    rb_ps = bcast_row(rstd[:], TPC)
    crb = pin.tile([128, TPC], BF16, tag="pinb", name="cq_rb")
    nc.vector.tensor_copy(crb[:], rb_ps[:])
    cqT = cq_raw
    for c in range(KC):
        u = tmp.tile([128, TPC], BF16, tag="bft", name="cq_u")
        nc.vector.tensor_mul(u[:], cq_raw[:, c, :], crb[:])
        nc.scalar.activation(cqT[:, c, :], u[:], ACTF.Identity,
                             scale=vcol("cnqw", c))

    # ================= stage 12: load gathered cross k/v (reuse q/v slots)
    ckb_sb = raw.tile([128, KC, CTX], BF16, tag="qr", name="ckb_sb")
    for r in range(CPB):
        nc.sync.dma_start(
            ckb_sb[:, slice(r * HC, (r + 1) * HC), :],
            bass.AP(tensor=agout.tensor,
                    offset=agout.offset + r * BLK2,
                    ap=[[CTX, 128], [128 * CTX, HC], [1, CTX]]))
    cvb_sb = kvp.tile([128, CTC * CPB, HDW], BF16, tag="vsb", name="cvb_sb")
    for r in range(CPB):
        nc.sync.dma_start(
            cvb_sb[:, slice(r * CTC, (r + 1) * CTC), :],
            bass.AP(tensor=agout.tensor,
                    offset=agout.offset + r * BLK2 + SZ_CKB,
                    ap=[[HDW, 128], [128 * HDW, CTC], [1, HDW]]))

    # ================= stage 13: cross-attention (token-sharded, all heads)
    cattnT = raw.tile([128, KC, TPC], BF16, tag="kr", name="cattnT")
    for hh in range(NH):
        r, hl = hh // HC, hh % HC
        aps = psA.tile([128, TPC], F32, tag="mm", name="caps")
        dps = psN.tile([1, TPC], F32, tag="nsum", name="cdps")
        for j in range(CTC):
            sps = psA.tile([128, TPC], F32, tag="mm", name="csps")
            nc.tensor.matmul(sps[:], ckb_sb[:, r * HC + hl, ts(j, 128)],
                             cqT[:, hh, :],
                             start=True, stop=True, skip_group_check=True)
            ex = exq.tile([128, TPC], BF16, tag="ex", name="cex")
            nc.scalar.activation(ex[:], sps[:], ACTF.Exp, scale=SCL)
            nc.tensor.matmul(aps[:], cvb_sb[:, r * CTC + j, ts(hl, 128)],
                             ex[:],
                             start=(j == 0), stop=(j == CTC - 1),
                             skip_group_check=True)
            nc.tensor.matmul(dps[:], ones_c[:], ex[:],
                             start=(j == 0), stop=(j == CTC - 1),
                             skip_group_check=True)
        rec = sm.tile([1, TPC], F32, tag="s", name="crec")
        nc.vector.reciprocal(rec[:], dps[:])
        rb_ps = bcast_row(rec[:], TPC)
        rb = tmp.tile([128, TPC], F32, tag="f32t", name="carb")
        nc.vector.tensor_copy(rb[:], rb_ps[:])
        nc.vector.tensor_mul(cattnT[:, hh, :], aps[:], rb[:])

    # ================= stage 14: co-proj + residual (in place) -> x3
    for o in range(KC):
        wt = wp.tile([128, DIM], BF16, tag="w", name="cwot")
        nc.gpsimd.dma_start(wt[:], cwo[o])
        ps = psA.tile([128, TPC], F32, tag="mm", name="co_ps")
        for k in range(KC):
            nc.tensor.matmul(ps[:], wt[:, ts(k, 128)], cattnT[:, k, :],
                             start=(k == 0), stop=(k == KC - 1),
                             skip_group_check=True)
        t1 = tmp.tile([128, TPC], F32, tag="f32t", name="cot1")
        nc.vector.tensor_scalar_add(t1[:], ps[:], vcol("cob", o))
        nc.vector.tensor_add(xown[:, o, :], t1[:], xown[:, o, :])
    if stop_stage <= 8:
        return stop_dump([xown[:, c, :] for c in range(KC)])

    # ================= stage 15: ln2 -> h2; FFN + MoE
    h2 = hs.tile([128, KC, TPC], BF16, tag="hh", name="h2")
    layernorm_own(xown, h2, "s2p", "sh2", None, None)

    wallb = xa.tile([128, NE * TPC], F32, tag="xall", name="wallb")
    nc.sync.dma_start(
        wallb[:],
        bass.AP(tensor=wall_d.tensor, offset=wall_d.offset,
                ap=[[0, 128], [1, NE * TPC]]))

    ff = hp.tile([128, FC, TPC], BF16, tag="hff", name="ff")
    for o in range(FC):
        wt = wp.tile([128, DIM], BF16, tag="w", name="w1t")
        nc.gpsimd.dma_start(wt[:], w1[o])
        ps = psA.tile([128, TPC], F32, tag="mm", name="ffps")
        for k in range(KC):
            nc.tensor.matmul(ps[:], wt[:, ts(k, 128)], h2[:, k, :],
                             start=(k == 0), stop=(k == KC - 1),
                             skip_group_check=True)
        nc.scalar.activation(ff[:, o, :], ps[:], ACTF.Gelu_apprx_tanh,
                             bias=vcol("fb1", o))

    FH = FC // 3
    for o in range(KC):
        ps = psA.tile([128, TPC], F32, tag="mm", name="w2ps")
        for kh in range(3):
            w2t = wp2.tile([128, FH * 128], BF16, tag="w2b", name="w2t")
            nc.sync.dma_start(
                w2t[:], w2[o][:, ds(kh * FH * 128, FH * 128)])
            for k in range(FH):
                kk = kh * FH + k
                nc.tensor.matmul(ps[:], w2t[:, ts(k, 128)], ff[:, kk, :],
                                 start=(kk == 0), stop=(kk == FC - 1),
                                 skip_group_check=True)
        acc = tmp.tile([128, TPC], F32, tag="f32t", name="macc")
        nc.vector.tensor_scalar_add(acc[:], ps[:], vcol("b2", o))
        for e in range(NE):
            met = wp.tile([128, DIM], BF16, tag="w", name="moet")
            nc.sync.dma_start(met[:], moew[e, o])
            pse = psA.tile([128, TPC], F32, tag="mm", name="pse")
            for k in range(KC):
                nc.tensor.matmul(pse[:], met[:, ts(k, 128)], h2[:, k, :],
                                 start=(k == 0), stop=(k == KC - 1),
                                 skip_group_check=True)
            te = tmp.tile([128, TPC], F32, tag="f32t", name="te")
            nc.vector.scalar_tensor_tensor(te[:], pse[:], vcol(f"mb{e}", o),
                                           wallb[:, ts(e, TPC)],
                                           ALU.add, ALU.mult)
            acc2 = tmp.tile([128, TPC], F32, tag="f32t", name="macc2")
            nc.vector.tensor_add(acc2[:], acc[:], te[:])
            acc = acc2
        t1 = tmp.tile([128, TPC], F32, tag="f32t", name="yt1")
        nc.vector.tensor_scalar_mul(t1[:], acc[:], vcol("g2", o))
        yc = tmp.tile([128, TPC], F32, tag="f32t", name="yc")
        nc.vector.tensor_add(yc[:], t1[:], xown[:, o, :])
        nc.sync.dma_start(yT[ts(o, 128), :], yc[:])

    ctx.close()


# -------------------------------------------------------------- host prep
def _rope_perm():
    p = np.arange(DIM).reshape(NH, HD)
    return np.concatenate([p[:, 0::2], p[:, 1::2]], axis=1).reshape(-1)


def _tile_oT(wm):
    """[IN, OUT] -> [OUT//128, 128, IN]; [o,p,k*128+j] = wm[k*128+p, o*128+j]."""
    IN, OUT = wm.shape
    ki, ko = IN // 128, OUT // 128
    return np.ascontiguousarray(
        wm.reshape(ki, 128, ko, 128).transpose(2, 1, 0, 3).reshape(ko, 128, IN))


def prep_inputs(inputs):
    f = lambda a: np.asarray(a, dtype=np.float32)
    x = f(inputs["x"])
    context = f(inputs["context"])
    t_mod = f(inputs["t_mod"])
    freqs_cos = f(inputs["freqs_cos"])
    freqs_sin = f(inputs["freqs_sin"])
    ew = f(inputs["expert_weights"])
    idx = np.asarray(inputs["top_k_indices"])
    modulation = f(inputs["modulation"])

    perm = _rope_perm()

    def wT(a):
        return np.ascontiguousarray(f(a).T).astype(bfnp)

    wq_t = _tile_oT(np.ascontiguousarray(f(inputs["sa_q_w"])[perm].T).astype(bfnp))
    wk_t = _tile_oT(np.ascontiguousarray(f(inputs["sa_k_w"])[perm].T).astype(bfnp))
    wv_h = wT(inputs["sa_v_w"])            # [IN, OUT]
    wo_t = _tile_oT(wT(inputs["sa_o_w"]))  # [KC,128,DIM(in)]
    cwq_t = _tile_oT(wT(inputs["ca_q_w"]))
    cwk_t = _tile_oT(wT(inputs["ca_k_w"]))
    cwv_h = wT(inputs["ca_v_w"])
    cwo_t = _tile_oT(wT(inputs["ca_o_w"]))
    w1_t = _tile_oT(wT(inputs["ffn_w1"]))
    w2_t = _tile_oT(wT(inputs["ffn_w2"]))
    moew_h = np.ascontiguousarray(f(inputs["moe_w"]).transpose(0, 2, 1)).astype(bfnp)
    moew_t = np.ascontiguousarray(
        np.stack([_tile_oT(moew_h[e]) for e in range(NE)]))
    moeb = f(inputs["moe_b"])

    mod = modulation + t_mod
    cosA = np.concatenate([freqs_cos.T, freqs_cos.T], 0).astype(bfnp)   # [128,S]
    sinA = np.concatenate([-freqs_sin.T, freqs_sin.T], 0).astype(bfnp)

    qb_p = f(inputs["sa_q_b"])[perm]
    kb_p = f(inputs["sa_k_b"])[perm]
    nqw_p = f(inputs["sa_nq_w"])[perm]
    nkw_p = f(inputs["sa_nk_w"])[perm]

    in_maps, metas = [], []
    for c in range(N_CORES):
        b, r = c // CPB, c % CPB
        tok = slice(r * TPC, (r + 1) * TPC)
        hsl = slice(r * HDW, (r + 1) * HDW)     # head-shard dims
        hcs = slice(r * HC, (r + 1) * HC)       # head-shard chunks
        vecs = np.zeros((128, NV), np.float32)

        def setv(name, arr):
            n = len(arr) // 128
            vecs[:, _VBASE[name]:_VBASE[name] + n] = arr.reshape(n, 128).T

        m = mod[b]
        setv("s1p", 1.0 + m[1]); setv("sh1", m[0]); setv("g1", m[2])
        setv("s2p", 1.0 + m[4]); setv("sh2", m[3]); setv("g2", m[5])
        setv("qb", qb_p[hsl])
        setv("kb", kb_p[hsl])
        setv("nqw", nqw_p[hsl])
        setv("nkw", nkw_p[hsl])
        setv("ob", f(inputs["sa_o_b"]))
        setv("cqb", f(inputs["ca_q_b"]))
        setv("ckb", f(inputs["ca_k_b"])[hsl])
        setv("cnqw", f(inputs["ca_nq_w"]))
        setv("cnkw", f(inputs["ca_nk_w"])[hsl])
        setv("cob", f(inputs["ca_o_b"]))
        setv("n3w", f(inputs["norm3_w"])); setv("n3b", f(inputs["norm3_b"]))
        setv("b2", f(inputs["ffn_b2"])); setv("fb1", f(inputs["ffn_b1"]))
        for e in range(NE):
            setv(f"mb{e}", moeb[e])

        rowb = np.concatenate([f(inputs["sa_v_b"])[hsl],
                               f(inputs["ca_v_b"])[hsl]])[None, :]

        wall = np.zeros((NE, TPC), np.float32)
        iw = idx[b, tok]
        eww = ew[b, tok]
        for kk in range(TOPK):
            np.add.at(wall, (iw[:, kk], np.arange(TPC)), eww[:, kk])

        in_maps.append({
            "xallT": np.ascontiguousarray(x[b].T).astype(bfnp),
            "xT": np.ascontiguousarray(x[b, tok].T),
            "ctxT": np.ascontiguousarray(context[b].T).astype(bfnp),
            "cosT": cosA,
            "sinT": sinA,
            "vecs": vecs,
            "rowb": rowb.astype(np.float32),
            "wq": np.ascontiguousarray(wq_t[hcs]),
            "wk": np.ascontiguousarray(wk_t[hcs]),
            "wv": np.ascontiguousarray(wv_h[:, hsl]),
            "wo": np.ascontiguousarray(wo_t[:, :, hsl]),
            "cwq": cwq_t,
            "cwk": np.ascontiguousarray(cwk_t[hcs]),
            "cwv": np.ascontiguousarray(cwv_h[:, hsl]),
            "cwo": cwo_t,
            "w1": w1_t, "w2": w2_t, "moew": moew_t,
            "wall": wall.astype(np.float32),
        })
        metas.append((b, r))
    return in_maps, metas


_NC_CACHE = {}


def get_nc(debug=False, stop_stage=99):
    key = (bool(debug), stop_stage)
    if key not in _NC_CACHE:
        _NC_CACHE[key] = build_bass(debug=debug, stop_stage=stop_stage)
    return _NC_CACHE[key]


def run(in_maps, debug=False, stop_stage=99):
    nc = get_nc(debug=debug, stop_stage=stop_stage)
    return bass_utils.run_bass_kernel_spmd(
        nc, in_maps, core_ids=list(range(N_CORES)), trace=False)


def kernel(**inputs):
    in_maps, metas = prep_inputs(inputs)
    res = run(in_maps, debug=bool(int(os.environ.get("BASSDIT_DEBUG", "0"))),
              stop_stage=int(os.environ.get("BASSDIT_STOP", "99")))
    out = np.zeros((B, S, DIM), np.float32)
    for c in range(N_CORES):
        b, r = metas[c]
        out[b, r * TPC:(r + 1) * TPC] = np.asarray(
            res.results[c]["yT"], dtype=np.float32).T
    kernel.last_results = res
    return out
